# revision 1
# baseline (speedup 1.0000x reference)
"""Bidirectional Mamba block on 8 Trainium2 NeuronCores.

Sharding: core = (batch b in 2) x (direction d in 2) x (d_inner half h in 2).
Every core runs the same SPMD Bass program on its (b, d) sequence with its
half of d_inner; the xi in-projection / conv / x-proj path is replicated
inside the (b, d) pair so no cross-core communication is needed.  The host
pre-transposes / pre-casts weights (layout only), pre-flips x for the
backward direction, and sums the two half-channel partial outputs per
(b, d) plus the flipped backward output at the end.

Model dims (hardcoded): B=2, L=1024, D_MODEL=1024, D_INNER=2048, N=16,
D_CONV=4, DT_RANK=64.
"""

import numpy as np
import ml_dtypes

B_SZ, SEQ = 2, 1024
D_MODEL, D_STATE, D_CONV = 1024, 16, 4
D_INNER = 2048
DT_RANK = 64
HALF = D_INNER // 2          # 1024 channels per core
NG_DM = D_MODEL // 128       # 8 partition groups over d_model
NG_CH = HALF // 128          # 8 partition groups over own channels
NG_XI = D_INNER // 128       # 16 partition groups over full xi channels
NPROJ = DT_RANK + 2 * D_STATE  # 96
T = SEQ
TH = T // 2                  # 512 (psum free-dim limit)
NQ = 4                       # state quarters for the scan
SQ = D_STATE // NQ           # 4 states per quarter
EPS = 1e-5

_BF16 = ml_dtypes.bfloat16

_CACHED = {}


def _build_nc():
    import concourse.bass as bass
    import concourse.tile as tile
    from concourse import bacc, mybir
    from concourse.masks import make_identity

    # Restrict ACT table-set choice to the two sets this kernel needs
    # (natural_log_exp_and_others covers Exp+Ln+Copy; silu_and_others covers
    # Silu).  The default chooser pairs Exp with exp_and_others and Ln with
    # natural_log, forcing a ~1.3us table reload around every softplus.
    if not getattr(bacc, "_act_tables_patched", False):
        from concourse import hw_specs as _hw
        _orig_tables = _hw.get_activation_tables
        _KEEP = {"natural_log_exp_and_others", "silu_and_others"}

        def _tables(arch):
            full = _orig_tables(arch)
            return {k: (v if k in _KEEP else set()) for k, v in full.items()}

        bacc.get_activation_tables = _tables
        bacc._act_tables_patched = True

    f32 = mybir.dt.float32
    bf16 = mybir.dt.bfloat16
    MULT = mybir.AluOpType.mult
    ADD = mybir.AluOpType.add
    AF = mybir.ActivationFunctionType

    nc = bacc.Bacc(num_devices=8)

    # ---- I/O ----
    xT = nc.declare_dram_parameter("xT", [D_MODEL, T], f32, isOutput=False)
    w_in_T = nc.declare_dram_parameter("w_in_T", [D_MODEL, 2 * HALF], bf16, isOutput=False)
    conv_w = nc.declare_dram_parameter("conv_w", [128, NG_CH, D_CONV], f32, isOutput=False)
    conv_b = nc.declare_dram_parameter("conv_b", [128, NG_CH, 1], f32, isOutput=False)
    xproj_wT = nc.declare_dram_parameter("xproj_wT", [128, NG_CH, NPROJ], bf16, isOutput=False)
    dt_wT = nc.declare_dram_parameter("dt_wT", [DT_RANK, HALF], bf16, isOutput=False)
    dt_b = nc.declare_dram_parameter("dt_b", [128, NG_CH, 1], f32, isOutput=False)
    Aneg = nc.declare_dram_parameter("Aneg", [128, NG_CH, D_STATE], f32, isOutput=False)
    D_skip = nc.declare_dram_parameter("D_skip", [128, NG_CH, 1], f32, isOutput=False)
    out_wT = nc.declare_dram_parameter("out_wT", [HALF, D_MODEL], bf16, isOutput=False)
    ln_g = nc.declare_dram_parameter("ln_g", [128, NG_DM, 1], f32, isOutput=False)
    ln_b = nc.declare_dram_parameter("ln_b", [128, NG_DM, 1], f32, isOutput=False)
    outT = nc.declare_dram_parameter("outT", [D_MODEL, T], f32, isOutput=True)

    TC = TH          # 512-column time chunks
    NCHUNK = T // TC

    st = {}

    def phase_consts(consts):
        ident = consts.tile([128, 128], bf16)
        make_identity(nc, ident[:])
        ones_col = consts.tile([128, 1], bf16)
        nc.vector.memset(ones_col[:], 1.0)
        eps_col = consts.tile([1, 1], f32)
        nc.vector.memset(eps_col[:], EPS)
        one_col = consts.tile([128, 1], f32)
        nc.vector.memset(one_col[:], 1.0)

        ln_g_sb = consts.tile([128, NG_DM, 1], f32)
        nc.sync.dma_start(ln_g_sb[:], ln_g[:])
        ln_b_sb = consts.tile([128, NG_DM, 1], f32)
        nc.sync.dma_start(ln_b_sb[:], ln_b[:])
        cw_sb = consts.tile([128, NG_CH, D_CONV], f32)
        nc.sync.dma_start(cw_sb[:], conv_w[:])
        cb_sb = consts.tile([128, NG_CH, 1], f32)
        nc.sync.dma_start(cb_sb[:], conv_b[:])
        dtb_col = consts.tile([128, NG_CH, 1], f32)
        nc.sync.dma_start(dtb_col[:], dt_b[:])
        A_sb = consts.tile([128, NG_CH, D_STATE], f32)
        nc.sync.dma_start(A_sb[:], Aneg[:])
        D_sb = consts.tile([128, NG_CH, 1], f32)
        nc.sync.dma_start(D_sb[:], D_skip[:])
        dtw_sb = consts.tile([DT_RANK, HALF], bf16)
        nc.sync.dma_start(dtw_sb[:], dt_wT[:])
        xpw_sb = consts.tile([128, NG_CH, NPROJ], bf16)
        nc.sync.dma_start(xpw_sb[:], xproj_wT[:])
        st.update(ident=ident, ones_col=ones_col, eps_col=eps_col, one_col=one_col,
                  ln_g_sb=ln_g_sb, ln_b_sb=ln_b_sb, cw_sb=cw_sb, cb_sb=cb_sb,
                  dtb_col=dtb_col, A_sb=A_sb, D_sb=D_sb, dtw_sb=dtw_sb,
                  xpw_sb=xpw_sb)

    def ln_chunk(c, pools):
        """LayerNorm for time columns [c*TC, (c+1)*TC) -> xnb tiles (bf16)."""
        lo = c * TC
        xnb_tiles = []
        xb_tiles = []
        for g in range(NG_DM):
            xb_g = pools["xbp"].tile([128, TC], bf16, tag="xb")
            # gpsimd software-DGE DMA casts f32 -> bf16 in flight
            nc.gpsimd.dma_start(xb_g[:], xT[g * 128:(g + 1) * 128, lo:lo + TC])
            xb_tiles.append(xb_g)

        stat_ps = []
        for which in range(2):
            ps = pools["psum"].tile([1, TC], f32, tag="mm")
            for g in range(NG_DM):
                if which == 0:
                    rhs = xb_tiles[g][:]
                else:
                    sq = pools["lns"].tile([128, TC], bf16, tag="sq")
                    nc.vector.tensor_mul(sq[:], xb_tiles[g][:], xb_tiles[g][:])
                    rhs = sq[:]
                nc.tensor.matmul(ps[:], st["ones_col"][:], rhs,
                                 start=(g == 0), stop=(g == NG_DM - 1))
            stat_ps.append(ps)

        mean_sb = pools["lns"].tile([1, TC], f32, tag="statrow")
        rstd_sb = pools["lns"].tile([1, TC], f32, tag="statrow")
        m2 = pools["lns"].tile([1, TC], f32, tag="statrow")
        nc.scalar.activation(mean_sb[:], stat_ps[0][:], AF.Copy, scale=1.0 / D_MODEL)
        nc.scalar.activation(rstd_sb[:], stat_ps[1][:], AF.Copy, scale=1.0 / D_MODEL)
        nc.vector.tensor_mul(m2[:], mean_sb[:], mean_sb[:])
        nc.vector.tensor_sub(rstd_sb[:], rstd_sb[:], m2[:])
        nc.scalar.activation(rstd_sb[:], rstd_sb[:], AF.Ln, bias=st["eps_col"][:])
        nc.scalar.activation(rstd_sb[:], rstd_sb[:], AF.Exp, scale=-0.5)

        mr_scr = pools["dram"].tile([2, TC], f32, tag="mr")
        nc.sync.dma_start(mr_scr[0:1, :], mean_sb[:])
        nc.sync.dma_start(mr_scr[1:2, :], rstd_sb[:])
        mean_bc = pools["lnbc"].tile([128, TC], bf16, tag="meanbc")
        rstd_bc = pools["lnbc"].tile([128, TC], bf16, tag="rstdbc")
        for i, dst in enumerate((mean_bc, rstd_bc)):
            srcap = bass.AP(tensor=mr_scr[:].tensor,
                            offset=mr_scr[:].offset + i * TC,
                            ap=[[0, 128], [1, TC]])
            nc.gpsimd.dma_start(dst[:], srcap)

        for g in range(NG_DM):
            t0 = pools["lns"].tile([128, TC], bf16, tag="lnt")
            nc.vector.tensor_sub(t0[:], xb_tiles[g][:], mean_bc[:])
            nc.vector.tensor_mul(t0[:], t0[:], rstd_bc[:])
            xnb_g = pools["xnbp"].tile([128, TC], bf16, tag="xnb")
            nc.vector.tensor_scalar(out=xnb_g[:], in0=t0[:],
                                    scalar1=st["ln_g_sb"][:, g, :],
                                    scalar2=st["ln_b_sb"][:, g, :],
                                    op0=MULT, op1=ADD)
            xnb_tiles.append(xnb_g)
        return xnb_tiles

    def mid_chunk_gen(c, pools, xnb_tiles, prev_tails, result):
        """in-proj + conv/silu + xproj for chunk c.

        The conv taps read the in-projection result straight out of PSUM
        (xi is never materialised); only a 3-column tail is carried to the
        next chunk."""
        pad = D_CONV - 1
        zs_tiles = []
        xc_tiles = []
        tails = []
        acc_list = []
        for oc in range(2 * NG_CH):
            ps = pools["psum"].tile([128, TC], f32, tag="mm")
            for g in range(NG_DM):
                wt = pools["wtp"].tile([128, 128], bf16, tag="wt")
                nc.sync.dma_start(
                    wt[:], w_in_T[g * 128:(g + 1) * 128, oc * 128:(oc + 1) * 128])
                nc.tensor.matmul(ps[:], wt[:], xnb_tiles[g][:],
                                 start=(g == 0), stop=(g == NG_DM - 1))
            if oc >= NG_CH:
                zg = pools["zp"].tile([128, TC], bf16, tag="z")
                nc.scalar.copy(zg[:], ps[:])
                zs_tiles.append(zg)
                yield
                continue
            # conv taps from PSUM: xc_pre[j] = sum_k w_k * xi[j + k - 3]
            taps = []
            for k in range(D_CONV):
                tk = pools["mids"].tile([128, TC], bf16, tag=f"tap{k}")
                nc.scalar.activation(tk[:, pad - k:TC], ps[:, 0:TC - pad + k],
                                     AF.Copy, scale=st["cw_sb"][:, oc, k:k + 1])
                if k < pad:
                    if c == 0:
                        nc.vector.memset(tk[:, 0:pad - k], 0.0)
                    else:
                        nc.scalar.activation(
                            tk[:, 0:pad - k], prev_tails[oc][:, k:pad],
                            AF.Copy, scale=st["cw_sb"][:, oc, k:k + 1])
                taps.append(tk)
            tail = pools["tailp"].tile([128, pad], bf16, tag="tail",
                                       name=f"tail_{c}_{oc}")
            nc.scalar.copy(tail[:], ps[:, TC - pad:TC])
            tails.append(tail)
            cps = pools["psum"].tile([128, TC], f32, tag="mm")
            for k in range(D_CONV):
                nc.tensor.matmul(cps[:], st["ident"][:], taps[k][:],
                                 start=(k == 0), stop=(k == D_CONV - 1))
            if c == 0:
                xc_g = pools["xcp"].tile([128, TC], bf16, tag="xc")
                nc.scalar.activation(xc_g[:], cps[:], AF.Silu,
                                     bias=st["cb_sb"][:, oc, :])
                xc_tiles.append(xc_g)
            else:
                acc = pools["mids"].tile([128, TC], bf16, tag="convacc")
                nc.scalar.copy(acc[:], cps[:])
                acc_list.append(acc)
            yield
        # batch remaining Silu ops contiguously to avoid ACT table thrash
        yield
        for zg in zs_tiles:
            nc.scalar.activation(zg[:], zg[:], AF.Silu)
        if c > 0:
            for oc in range(NG_CH):
                xc_g = pools["xcp"].tile([128, TC], bf16, tag="xc")
                nc.scalar.activation(xc_g[:], acc_list[oc][:], AF.Silu,
                                     bias=st["cb_sb"][:, oc, :])
                xc_tiles.append(xc_g)
        yield

        ps = pools["psum"].tile([NPROJ, TC], f32, tag="mm")
        for oc in range(NG_CH):
            nc.tensor.matmul(ps[:], st["xpw_sb"][:, oc, :], xc_tiles[oc][:],
                             start=(oc == 0), stop=(oc == NG_CH - 1))
        dbl_part = pools["mids"].tile([NPROJ, TC], bf16, tag="dblp")
        nc.scalar.copy(dbl_part[:], ps[:])
        dbl_in = pools["dram"].tile([NPROJ, TC], bf16, tag="dbl_in")
        dbl_out = pools["dram"].tile([NPROJ, TC], bf16, tag="dbl_out")
        nc.sync.dma_start(dbl_in[:], dbl_part[:])
        nc.gpsimd.collective_compute(
            "AllReduce", mybir.AluOpType.add,
            replica_groups=[[0, 1], [2, 3], [4, 5], [6, 7]],
            ins=[dbl_in[:]], outs=[dbl_out[:]])
        dtb_sb = pools["dtbp"].tile([DT_RANK, TC], bf16, tag="dtb")
        nc.sync.dma_start(dtb_sb[:], dbl_out[0:DT_RANK, :])

        result.extend([xc_tiles, dtb_sb, dbl_out, zs_tiles, tails])

    def bc_broadcast(pools, dbl_out):
        B_rep = pools["brep"].tile([128, D_STATE, TC], bf16, tag="Brep")
        C_rep = pools["brep"].tile([128, D_STATE, TC], bf16, tag="Crep")
        for i, dst in enumerate((B_rep, C_rep)):
            for n in range(D_STATE):
                srcap = bass.AP(tensor=dbl_out[:].tensor,
                                offset=dbl_out[:].offset
                                + (DT_RANK + i * D_STATE + n) * TC,
                                ap=[[0, 128], [1, TC]])
                nc.sync.dma_start(dst[:, n, :], srcap)
        return B_rep, C_rep

    def scan_chunk_gen(c, pools, xc_tiles, dtb_sb, B_rep, C_rep, zs_tiles,
                       carries, yg_tiles):
        for g in range(NG_CH):
            delta_g = pools["dup"].tile([128, TC], bf16, tag="delta")
            dps = pools["psum"].tile([128, TC], f32, tag="mm")
            nc.tensor.matmul(dps[:], st["dtw_sb"][:, g * 128:(g + 1) * 128],
                             dtb_sb[:], start=True, stop=True)
            nc.scalar.activation(delta_g[:], dps[:], AF.Exp,
                                 bias=st["dtb_col"][:, g, :])
            nc.scalar.activation(delta_g[:], delta_g[:], AF.Ln,
                                 bias=st["one_col"][:])
            u_rep = pools["urep"].tile([128, SQ, TC], bf16, tag="urep")
            nc.vector.tensor_mul(u_rep[:, 0, :], delta_g[:], xc_tiles[g][:])
            rep_src = bass.AP(tensor=u_rep[:].tensor, offset=u_rep[:].offset,
                              ap=[u_rep[:].ap[0], [0, SQ - 1], [1, TC]])
            nc.sync.dma_start(u_rep[:, 1:SQ, :], rep_src)

            y_ps = pools["ypsum"].tile([128, TC], f32, tag="y")
            for q in range(NQ):
                dA = pools["p_da"].tile([128, SQ, TC], bf16, tag="dA")
                for j in range(SQ):
                    nc.scalar.activation(
                        dA[:, j, :], delta_g[:], AF.Exp,
                        scale=st["A_sb"][:, g, q * SQ + j: q * SQ + j + 1])
                dBu = pools["p_dbu"].tile([128, SQ, TC], bf16, tag="dBu")
                nc.vector.tensor_mul(
                    dBu[:].rearrange("p n t -> p (n t)"),
                    u_rep[:].rearrange("p n t -> p (n t)"),
                    B_rep[:, q * SQ:(q + 1) * SQ, :].rearrange("p n t -> p (n t)"))
                if c > 0:
                    # inject carried state: dBu[:, :, 0] += dA[:, :, 0] * carry
                    inj = pools["tiny"].tile([128, SQ, 1], bf16, tag="inj")
                    nc.vector.tensor_mul(inj[:], dA[:, :, 0:1], carries[g][q][:])
                    nc.vector.tensor_add(dBu[:, :, 0:1], dBu[:, :, 0:1], inj[:])
                # zero t=0 of every state block (no cross-block leakage)
                nc.vector.tensor_scalar_mul(dA[:, :, 0:1], dA[:, :, 0:1], 0.0)
                Hh = pools["p_h"].tile([128, SQ, TC], bf16, tag="H")
                nc.vector.tensor_tensor_scan(
                    out=Hh[:].rearrange("p n t -> p (n t)"),
                    data0=dA[:].rearrange("p n t -> p (n t)"),
                    data1=dBu[:].rearrange("p n t -> p (n t)"),
                    initial=0.0, op0=MULT, op1=ADD)
                if c + 1 < NCHUNK:
                    nc.vector.tensor_copy(carries[g][q][:], Hh[:, :, TC - 1:TC])
                # H *= C  (dense bf16 2x on DVE)
                nc.vector.tensor_mul(
                    Hh[:].rearrange("p n t -> p (n t)"),
                    Hh[:].rearrange("p n t -> p (n t)"),
                    C_rep[:, q * SQ:(q + 1) * SQ, :].rearrange("p n t -> p (n t)"))
                for j in range(SQ):
                    nc.tensor.matmul(y_ps[:], st["ident"][:], Hh[:, j, :],
                                     start=(q == 0 and j == 0),
                                     stop=(q == NQ - 1 and j == SQ - 1))

            yd = pools["mids"].tile([128, TC], bf16, tag="yd")
            nc.vector.scalar_tensor_tensor(
                out=yd[:], in0=xc_tiles[g][:], scalar=st["D_sb"][:, g, :],
                in1=y_ps[:], op0=MULT, op1=ADD)
            yg_g = pools["ygp"].tile([128, TC], bf16, tag="yg")
            nc.vector.tensor_mul(yg_g[:], yd[:], zs_tiles[g][:])
            yg_tiles.append(yg_g)
            yield

    def out_chunk_gen(c, pools, yg_tiles):
        # g-outer accumulation: the output matmuls start as soon as each
        # yg tile lands instead of waiting for the whole scan to finish.
        lo = c * TC
        MH = NG_DM // 2
        for half in range(2):
            opss = []
            for mi in range(MH):
                ops_t = pools["psum"].tile([128, TC], f32, tag="mm",
                                           name=f"ops_{c}_{half}_{mi}")
                opss.append(ops_t)
            for g in range(NG_CH):
                for mi in range(MH):
                    m = half * MH + mi
                    wt = pools["wtp"].tile([128, 128], bf16, tag="owt")
                    nc.sync.dma_start(
                        wt[:], out_wT[g * 128:(g + 1) * 128,
                                      m * 128:(m + 1) * 128])
                    nc.tensor.matmul(opss[mi][:], wt[:], yg_tiles[g][:],
                                     start=(g == 0), stop=(g == NG_CH - 1))
                yield
            for mi in range(MH):
                m = half * MH + mi
                osb = pools["outp"].tile([128, TC], f32, tag="osb")
                nc.scalar.copy(osb[:], opss[mi][:])
                nc.sync.dma_start(outT[m * 128:(m + 1) * 128, lo:lo + TC],
                                  osb[:])

    from contextlib import ExitStack

    with ExitStack() as stack:
        tc = stack.enter_context(tile.TileContext(nc))
        ep = stack.enter_context
        pools = dict(
            consts=ep(tc.tile_pool(name="consts", bufs=1)),
            dram=ep(tc.tile_pool(name="dram", bufs=2, space="DRAM")),
            psum=ep(tc.tile_pool(name="psum", bufs=6, space="PSUM")),
            ypsum=ep(tc.tile_pool(name="ypsum", bufs=2, space="PSUM")),
            xbp=ep(tc.tile_pool(name="xbp", bufs=13)),
            lns=ep(tc.tile_pool(name="lns", bufs=3)),
            lnbc=ep(tc.tile_pool(name="lnbc", bufs=2)),
            xnbp=ep(tc.tile_pool(name="xnbp", bufs=14)),
            wtp=ep(tc.tile_pool(name="wtp", bufs=16)),
            tailp=ep(tc.tile_pool(name="tailp", bufs=2 * NG_XI)),
            xcp=ep(tc.tile_pool(name="xcp", bufs=16)),
            zp=ep(tc.tile_pool(name="zp", bufs=16)),
            mids=ep(tc.tile_pool(name="mids", bufs=3)),
            dtbp=ep(tc.tile_pool(name="dtbp", bufs=2)),
            brep=ep(tc.tile_pool(name="brep", bufs=1)),
            dup=ep(tc.tile_pool(name="dup", bufs=3)),
            urep=ep(tc.tile_pool(name="urep", bufs=2)),
            p_da=ep(tc.tile_pool(name="p_da", bufs=2)),
            p_dbu=ep(tc.tile_pool(name="p_dbu", bufs=2)),
            p_h=ep(tc.tile_pool(name="p_h", bufs=2)),
            tiny=ep(tc.tile_pool(name="tiny", bufs=4)),
            carryp=ep(tc.tile_pool(name="carryp", bufs=NG_CH * NQ)),
            ygp=ep(tc.tile_pool(name="ygp", bufs=9)),
            outp=ep(tc.tile_pool(name="outp", bufs=2)),
        )
        if True:
            if True:
                phase_consts(pools["consts"])

                carryp = pools["carryp"]
                # carry state tiles across chunks
                carries = []
                for _g in range(NG_CH):
                    row = []
                    for _q in range(NQ):
                        cr = carryp.tile([128, SQ, 1], bf16, tag="carry",
                                         name=f"carry_{_g}_{_q}")
                        row.append(cr)
                    carries.append(row)

                def drive(*gens_ratio):
                    """Round-robin generators: list of (gen, weight)."""
                    active = [[g, w] for g, w in gens_ratio]
                    while active:
                        for gw in list(active):
                            g, w = gw
                            for _ in range(w):
                                try:
                                    next(g)
                                except StopIteration:
                                    active.remove(gw)
                                    break

                # pipeline: ln0 | mid0+ln1 | scan0+mid1 | out0+scan1 | out1
                xnb0 = ln_chunk(0, pools)
                m0 = []
                gm0 = mid_chunk_gen(0, pools, xnb0, None, m0)
                # run first few mid0 units, then emit ln1 alongside
                for _ in range(6):
                    next(gm0)
                xnb1 = ln_chunk(1, pools)
                for _ in gm0:
                    pass
                xc0, dtb0, bcs0, zs0, tails0 = m0
                B0, C0 = bc_broadcast(pools, bcs0)

                yg0 = []
                gs0 = scan_chunk_gen(0, pools, xc0, dtb0, B0, C0, zs0,
                                     carries, yg0)
                m1 = []
                gm1 = mid_chunk_gen(1, pools, xnb1, tails0, m1)
                drive((gs0, 1), (gm1, 3))
                xc1, dtb1, bcs1, zs1, _ = m1
                B1, C1 = bc_broadcast(pools, bcs1)

                yg1 = []
                gs1 = scan_chunk_gen(1, pools, xc1, dtb1, B1, C1, zs1,
                                     carries, yg1)
                go0 = out_chunk_gen(0, pools, yg0)
                drive((gs1, 1), (go0, 1))
                for _ in out_chunk_gen(1, pools, yg1):
                    pass

    nc.finalize()
    return nc


def _shard_inputs(inputs):
    """Build the 8 per-core input maps from the full-problem inputs."""
    x = np.asarray(inputs["x"], np.float32)
    in_maps = []
    for core in range(8):
        b = core // 4
        d = (core // 2) % 2
        h = core % 2
        p = "f_" if d == 0 else "b_"
        in_w = np.asarray(inputs[p + "in_w"], np.float32)
        conv_w = np.asarray(inputs[p + "conv_w"], np.float32)
        conv_b = np.asarray(inputs[p + "conv_b"], np.float32)
        xproj_w = np.asarray(inputs[p + "xproj_w"], np.float32)
        dt_w = np.asarray(inputs[p + "dt_w"], np.float32)
        dt_b = np.asarray(inputs[p + "dt_b"], np.float32)
        A_log = np.asarray(inputs[p + "A_log"], np.float32)
        D_sk = np.asarray(inputs[p + "D_skip"], np.float32)
        out_w = np.asarray(inputs[p + "out_w"], np.float32)

        xb = x[b]
        if d == 1:
            xb = xb[::-1]

        own = slice(h * HALF, (h + 1) * HALF)
        w_xi = in_w[:D_INNER][own]                # (1024, 1024) own half of xi
        w_z = in_w[D_INNER:][own]                 # (1024, 1024) own half of z
        w_in_T = np.concatenate([w_xi.T, w_z.T], axis=1)  # (1024, 2048)

        def grp(a, ng):
            k = a.shape[1] if a.ndim > 1 else 1
            return np.ascontiguousarray(
                a.reshape(ng, 128, k).transpose(1, 0, 2))

        m = {
            "xT": np.ascontiguousarray(xb.T),
            "w_in_T": np.ascontiguousarray(w_in_T).astype(_BF16),
            "conv_w": grp(conv_w[own], NG_CH),
            "conv_b": grp(conv_b[own], NG_CH),
            "xproj_wT": grp(xproj_w[:, own].T, NG_CH).astype(_BF16),
            "dt_wT": np.ascontiguousarray(dt_w[own].T).astype(_BF16),
            "dt_b": grp(dt_b[own], NG_CH),
            "Aneg": grp(-np.exp(A_log[own]), NG_CH),
            "D_skip": grp(D_sk[own], NG_CH),
            "out_wT": np.ascontiguousarray(0.5 * out_w[:, own].T).astype(_BF16),
            "ln_g": grp(np.asarray(inputs["ln_g"], np.float32), NG_DM),
            "ln_b": grp(np.asarray(inputs["ln_b"], np.float32), NG_DM),
        }
        in_maps.append(m)
    return in_maps


def kernel(**inputs):
    # If tracing is requested via env but the runtime image lacks
    # antenv.axon_hooks, register a stub so run_bass_kernel_spmd degrades
    # gracefully instead of crashing on import.
    import sys as _sys
    try:
        import antenv.axon_hooks  # noqa: F401
    except ImportError:
        import types as _types
        import antenv as _antenv
        _m = _types.ModuleType("antenv.axon_hooks")
        _m._hook = None
        _m.set_axon_ntff_profile_hook = lambda h: setattr(_m, "_hook", h)
        _m.get_axon_ntff_profile_hook = lambda: _m._hook
        _sys.modules["antenv.axon_hooks"] = _m
        _antenv.axon_hooks = _m

    from concourse.bass_utils import run_bass_kernel_spmd

    if "nc" not in _CACHED:
        _CACHED["nc"] = _build_nc()
    nc = _CACHED["nc"]

    in_maps = _shard_inputs(inputs)
    res = run_bass_kernel_spmd(nc, in_maps, core_ids=list(range(8)))
    _CACHED["last_res"] = res
    outs = [np.asarray(r["outT"], np.float32) for r in res.results]

    out = np.empty((B_SZ, SEQ, D_MODEL), np.float32)
    for b in range(B_SZ):
        fwd = (outs[b * 4 + 0] + outs[b * 4 + 1]).T          # (t, dm)
        bwd = (outs[b * 4 + 2] + outs[b * 4 + 3]).T[::-1]    # un-flip time
        out[b] = fwd + bwd
    return out

